# revision 42
# baseline (speedup 1.0000x reference)
"""Channel-attention (single-head shared attention over channels) Trainium2 kernel.

Reference computation (per batch b, C=512 channels, N=64*64=4096 spatial):
    xf = x[b].reshape(C, N)
    q = wq[:,None]*xf ; k = wk[:,None]*xf ; v = wv[:,None]*xf
    attn = softmax(q @ k.T / sqrt(N), axis=-1)        # (C, C)
    out[b] = (attn @ v).reshape(C, H, W)

Strategy (data-parallel over B across 8 cores, 2 batches/core):
  * Gram matrix in fp8 DoubleRow (0.5 cyc/row): x is cast RAW to fp8e4
    (values ~N(0,1) sit in e4m3's sweet spot; folding the tiny wq scale in,
    as a bf16 kernel would, lands in fp8 subnormals).  The transposed
    operands come from fp8 PE transposes (stride-2 PSUM writes per the ISA
    rule) compacted to stride-1 SBUF tiles laid out [n-pair, 2, C] so both
    DoubleRow APs are walrus-legal ([K, 2 (stride%16==0), M (stride 1)]).
  * Logits take the two-sided scale wk_d*wq_c/sqrt(N) from a host-computed
    rank-1 W2 matrix (DVE multiply) before the ACT exp -> E in bf16.
  * mm2 out = E @ v runs in float32r (full 1 cyc/row at 512-wide moving):
    the rhs is the raw staged f32 x (declared f32r, zero extra casts!) and
    the lhsT is E with wv folded per-partition (DVE, f32r out).  Z = sum E
    via ones-matmuls on the PE; the softmax 1/Z is applied as the per-
    partition scale of the PSUM->bf16 output downcast (ACT/DVE).
  * Output is written bf16 (halves the output DMA) and upcast on host.
  * Scheduling: engines execute strictly in-order, so PSUM-dependent ops
    (compaction) never share a queue with input-critical casts; batch 1's
    transposes+Gram are emitted inside batch 0's mm2 window; Z is emitted
    two mm2 groups in (the op-ring absorbs the downcast lag).
  Measured: 124413 ns TimelineSim (baseline 129427), rel err 2.3e-3.
"""

import numpy as np
import ml_dtypes

import concourse.bass as bass
import concourse.tile as tile
from concourse import mybir
from concourse.bass_utils import run_bass_kernel_spmd
from concourse.masks import make_identity

P = 128
C = 512
N = 4096
B_TOTAL = 16
N_CORES = 8
B_PER_CORE = B_TOTAL // N_CORES
CI = C // P          # 4 channel chunks
NCH = 8              # input staged in 512-n chunks
NPAIR = 16           # 256-n DoubleRow pairs per batch
F32 = mybir.dt.float32
F32R = mybir.dt.float32r
BF16 = mybir.dt.bfloat16
FP8 = mybir.dt.float8e4

MM2_FP8 = False      # bf16 mm2 (fp8 DoubleRow variant kept for reference)


def _split_multiwaits(nc):
    """Workaround: this walrus build rejects instructions carrying >1 sync
    wait ("Too many sync wait commands").  Hoist all but the last wait onto
    standalone EventSemaphore instructions placed just before the owner (same
    engine, so sequencer order preserves semantics)."""
    for f in nc.m.functions:
        for blk in f.blocks:
            new_insts = []
            for ins in blk.instructions:
                si = ins.sync_info
                if si is not None and si.on_wait is not None and len(si.on_wait) > 1:
                    waits = list(si.on_wait)
                    for k, w in enumerate(waits[:-1]):
                        new_insts.append(
                            mybir.InstEventSemaphore(
                                name=f"{ins.name}_splitw{k}",
                                engine=ins.engine,
                                sync_info=mybir.SyncInfo(on_wait=[w], on_update=[]),
                            )
                        )
                    si.on_wait = [waits[-1]]
                new_insts.append(ins)
            blk.instructions[:] = new_insts


def build_kernel():
    nc = bass.Bass()
    x_in = nc.dram_tensor("x", [B_PER_CORE, C, N], F32R, kind="ExternalInput")
    wvc_in = nc.dram_tensor("wvc", [P, CI], F32, kind="ExternalInput")
    w2_in = nc.dram_tensor("w2", [P, CI, C], F32, kind="ExternalInput")
    out = nc.dram_tensor("out", [B_PER_CORE, C, N], BF16, kind="ExternalOutput")

    with tile.TileContext(nc) as tc:
        with (
            tc.tile_pool(name="singles", bufs=1) as singles,
            
            tc.tile_pool(name="x8p", bufs=4) as x8p,
            tc.tile_pool(name="xtp", bufs=3) as xtp,
            tc.tile_pool(name="vp", bufs=2) as vp,
            tc.tile_pool(name="ep", bufs=2) as ep,
            tc.tile_pool(name="sp2", bufs=2) as sp2,
            tc.tile_pool(name="misc", bufs=4) as miscp,
            tc.tile_pool(name="rzp", bufs=8) as rzp,
            tc.tile_pool(name="osbp", bufs=3) as osbp,
            tc.tile_pool(name="gpp", bufs=4, space="PSUM") as gpp,
            tc.tile_pool(name="tpp", bufs=4, space="PSUM") as tpp,
        ):
            wvc = singles.tile([P, CI], F32, name="wvc")
            w2 = singles.tile([P, CI, C], F32, name="w2")
            nc.sync.dma_start(wvc, wvc_in[:, :])
            ident8 = singles.tile([P, P], FP8, name="ident8")
            make_identity(nc, ident8)
            identf = singles.tile([P, P], F32, name="identf")
            make_identity(nc, identf)
            onesb = singles.tile([P, 1], BF16, name="onesb")
            nc.gpsimd.memset(onesb, 1.0)

            # per-batch state (rings of 2)
            stgs, ets, etvs = {}, {}, {}
            x8s = {}          # (b, k) -> chunk tile
            xts = {}          # (b, k) -> transposed chunk tile
            gpss = {}         # b -> [gp tile per dc]
            rzs = {}          # (b, cc) -> 1/Z tile
            w2_loaded = [False]

            def nonpe_chunk(b, k, split_first=False):
                """Stage DMA + fp8/bf16 casts for chunk k of batch b."""
                nsl = slice(k * 512, (k + 1) * 512)
                xr = x_in[b].rearrange("(i p) n -> p i n", p=P)
                stg = stgs[b][:, :, nsl]
                x8 = x8p.tile([P, CI, 512], FP8, tag="x8", name=f"x8_{b}_{k}")
                x8s[(b, k)] = x8
                if split_first:
                    # first 256 n-cols land + cast early so the PE starts
                    # transposing ~2us sooner
                    nc.sync.dma_start(stg[:, :, :256], xr[:, :, 0:256])
                    nc.sync.dma_start(stg[:, :, 256:], xr[:, :, 256:512])
                    nc.gpsimd.tensor_copy(out=x8[:, :, :256], in_=stg[:, :, :256])
                    nc.gpsimd.tensor_copy(out=x8[:, :, 256:], in_=stg[:, :, 256:])
                elif k % 2 == 0:
                    nc.sync.dma_start(stg, xr[:, :, nsl])
                    nc.gpsimd.tensor_copy(out=x8[:, :, :], in_=stg[:, :, :])
                else:
                    nc.sync.dma_start(stg, xr[:, :, nsl])
                    nc.scalar.activation(
                        x8[:, :, :], stg[:, :, :],
                        func=mybir.ActivationFunctionType.Copy,
                    )
                if not w2_loaded[0]:
                    # W2 is first needed at phase3; load it after chunk 0
                    nc.sync.dma_start(w2, w2_in[:, :, :])
                    w2_loaded[0] = True

            def pe_tp_chunk(b, k):
                """fp8 transposes (stride-2 PSUM) + compaction for chunk k."""
                x8 = x8s.pop((b, k))
                xtu = xtp.tile([P, 2, 2, CI, P], FP8, tag="xtu", name=f"xtu_{b}_{k}")
                xts[(b, k)] = xtu
                for g2 in range(2):  # one 256-n pair per group
                    tpg = tpp.tile(
                        [P, 2048], FP8, tag="tpg", name=f"tpg_{b}_{k}_{g2}"
                    )
                    gsv = tpg.rearrange("p (q c two) -> p q c two", two=2, c=P)
                    for blk2 in range(2):
                        nb = g2 * 2 + blk2
                        for ci in range(CI):
                            nc.tensor.transpose(
                                gsv[:, blk2 * CI + ci, :, 0],
                                x8[:, ci, nb * P:(nb + 1) * P],
                                ident8,
                            )
                    # GPSIMD cannot read PSUM.  Batch 0's input phase is
                    # DMA-paced: DVE-only keeps ACT's x8 casts unblocked.
                    # Later batches run tp as a burst inside the previous
                    # mm2 window: split cmp across DVE+ACT (x8s are done).
                    cmp_out = xtu[:, g2].rearrange("p i ci c -> p (i ci c)")
                    cmp_in = gsv[:, :, :, 0].rearrange("p q c -> p (q c)")
                    if b == 0 or g2 == 0:
                        nc.vector.tensor_copy(out=cmp_out, in_=cmp_in)
                    else:
                        nc.scalar.activation(
                            cmp_out, cmp_in,
                            func=mybir.ActivationFunctionType.Copy,
                        )

            def pe_mm1_chunk(b, k):
                """DoubleRow Gram accumulation for the 2 pairs of chunk k.
                The last chunk runs dc-major so gp0 stops ~1us before gp3 and
                the exp chain overlaps the mm1 tail."""
                gps = gpss[b]
                xtu = xts.pop((b, k))
                if k == NCH - 1:
                    for dc in range(CI):
                        for g2 in range(2):
                            t = k * 2 + g2
                            _mm1_one(gps, xtu, g2, t, dc)
                    return
                for g2 in range(2):
                    t = k * 2 + g2
                    for dc in range(CI):
                        _mm1_one(gps, xtu, g2, t, dc)

            def _mm1_one(gps, xtu, g2, t, dc):
                if True:
                    if True:
                        lhsT = xtu[:, g2, :, dc, :]
                        rhs_all = xtu[:, g2].rearrange("p i ci c -> p i (ci c)")
                        off = 0
                        first_piece = True
                        while off < C:
                            w = min(256, C - off)
                            nc.tensor.matmul(
                                gps[dc][:, off:off + w],
                                lhsT=lhsT,
                                rhs=rhs_all[:, :, off:off + w],
                                start=(t == 0 and first_piece),
                                stop=(t == NPAIR - 1 and off + w == C),
                                perf_mode=mybir.MatmulPerfMode.DoubleRow,
                                skip_group_check=True,
                            )
                            off += w
                            first_piece = False
                        return

            def phase3(b):
                """W2 scale + exp (+mirrors), E splits, Z and reciprocals."""
                gps = gpss[b]
                et = ets[b]
                etv = etvs[b]
                for dc in range(CI):
                    s = sp2.tile([P, C], F32, tag="s", name=f"s_{b}_{dc}")
                    nc.vector.tensor_tensor(
                        out=s,
                        in0=gps[dc],
                        in1=w2[:, dc, :],
                        op=mybir.AluOpType.mult,
                    )
                    nc.scalar.activation(
                        et[:, dc, :], s,
                        func=mybir.ActivationFunctionType.Exp,
                    )
                    # fold wv into E per dc (Z uses the unfolded et tiles);
                    # f32r so the mm2 runs at full rate
                    nc.vector.tensor_scalar(
                        out=etv[:, dc, :], in0=et[:, dc, :],
                        scalar1=wvc[:, dc:dc + 1], scalar2=None,
                        op0=mybir.AluOpType.mult,
                    )
            def emit_z(b):
                """Z ones-matmuls + reciprocals, emitted a couple of mm2
                groups in so the PE never parks on the exp chain (the op
                ring absorbs the outscale lag)."""
                et = ets[b]
                for cc in range(CI):
                    zpt = gpp.tile([P, C], F32, tag="gp", name=f"zp_{b}_{cc}")
                    zp = zpt[:, 0:1]
                    for dc in range(CI):
                        nc.tensor.matmul(
                            zp,
                            lhsT=et[:, dc, cc * P:(cc + 1) * P],
                            rhs=onesb,
                            start=(dc == 0),
                            stop=(dc == CI - 1),
                        )
                    rz = rzp.tile([P, 1], F32, tag="rz", name=f"rz_{b}_{cc}")
                    rzs[(b, cc)] = rz
                    nc.vector.reciprocal(rz, zp)

            def mm2_group(b, cc, fine_tail=False, z_mid=False):
                """mm2 DoubleRow passes + 1/Z downcast + output DMA for cc."""
                csl = slice(cc * P, (cc + 1) * P)
                etv = etvs[b]
                xs = stgs[b]
                def one_nt_matmuls(nt):
                    op = gpp.tile([P, C], F32, tag="gp",
                                  name=f"op_{b}_{cc}_{nt}")[:, :512]
                    ntl = slice(nt * 512, (nt + 1) * 512)
                    for dc in range(CI):
                        nc.tensor.matmul(
                            op,
                            lhsT=etv[:, dc, csl],
                            rhs=xs[:, dc, ntl],
                            start=(dc == 0),
                            stop=(dc == CI - 1),
                        )
                    return op

                def one_nt_post(nt, op, osb, q):
                    # deferred softmax 1/Z as the PSUM->bf16 downcast
                    rz = rzs[(b, cc)]
                    if nt % 4 != 3:
                        nc.scalar.activation(
                            osb[:, q, :], op,
                            func=mybir.ActivationFunctionType.Copy,
                            scale=rz,
                        )
                    else:
                        nc.vector.tensor_scalar(
                            out=osb[:, q, :], in0=op,
                            scalar1=rz, scalar2=None,
                            op0=mybir.AluOpType.mult,
                        )
                    if fine_tail:
                        nc.scalar.dma_start(
                            out[b, csl, nt * 512:(nt + 1) * 512], osb[:, q, :]
                        )

                pending = []
                for nt2 in range(4):
                    osb = osbp.tile([P, 2, 512], BF16, tag="osb",
                                    name=f"osb_{b}_{cc}_{nt2}")
                    for q in range(2):
                        nt = nt2 * 2 + q
                        op = one_nt_matmuls(nt)
                        if z_mid and nt2 == 0:
                            # hold the downcasts until Z exists; the PE has
                            # these two groups as runway over the exp tail
                            pending.append((nt, op, osb, q))
                            continue
                        if z_mid and nt2 == 1 and pending:
                            emit_z(b)
                            for (pnt, pop, posb, pq) in pending:
                                one_nt_post(pnt, pop, posb, pq)
                            pending = []
                        one_nt_post(nt, op, osb, q)
                    if not fine_tail and not (z_mid and nt2 == 0):
                        nc.scalar.dma_start(
                            out[b, csl, nt2 * 1024:(nt2 + 1) * 1024], osb
                        )
                    elif not fine_tail and z_mid and nt2 == 0:
                        first_osb = osb
                if not fine_tail and z_mid:
                    nc.scalar.dma_start(out[b, csl, 0:1024], first_osb)

            # ---------------- emission schedule ----------------
            for b in range(B_PER_CORE):
                stgs[b] = vp.tile([P, CI, N], F32R, tag="stg", name=f"stg_{b}")
                ets[b] = ep.tile([P, CI, C], BF16, tag="et", name=f"et_{b}")
                etvs[b] = ep.tile([P, CI, C], F32R, tag="etv", name=f"etv_{b}")

                def alloc_gps():
                    gpss[b] = [
                        gpp.tile([P, C], F32, tag="gp", name=f"gp{dc}_{b}")
                        for dc in range(CI)
                    ]

                if b == 0:
                    alloc_gps()
                    for k in range(NCH):
                        nonpe_chunk(b, k, split_first=(k == 0))
                        pe_tp_chunk(b, k)
                        if k > 0:
                            pe_mm1_chunk(b, k - 1)
                    pe_mm1_chunk(b, NCH - 1)
                else:
                    # inputs of batch b interleave with mm2 of batch b-1;
                    # PE work (tp+mm1) is emitted after, so the tensor engine
                    # stream stays in-order and stall-free.
                    alloc_gps()
                    for cc in range(CI):
                        nonpe_chunk(b, 2 * cc)
                        nonpe_chunk(b, 2 * cc + 1)
                        mm2_group(b - 1, cc, z_mid=(cc == 0))
                        # fold this batch's transposes+Gram into the previous
                        # batch's mm2 window (PE stays busy either way; the
                        # dedicated tp phase vanishes from the critical path)
                        pe_tp_chunk(b, 2 * cc)
                        if cc > 0:
                            pe_mm1_chunk(b, 2 * cc - 1)
                        pe_tp_chunk(b, 2 * cc + 1)
                        pe_mm1_chunk(b, 2 * cc)
                    pe_mm1_chunk(b, NCH - 1)
                phase3(b)
            for cc in range(CI):
                mm2_group(B_PER_CORE - 1, cc, fine_tail=(cc >= 2), z_mid=(cc == 0))

    _split_multiwaits(nc)
    return nc


_NC_CACHE = None


def _get_nc():
    global _NC_CACHE
    if _NC_CACHE is None:
        _NC_CACHE = build_kernel()
    return _NC_CACHE


def make_weight_inputs(wq, wk, wv):
    wq = np.asarray(wq, np.float64)
    wk = np.asarray(wk, np.float64)
    wv = np.asarray(wv, np.float64)
    rn = np.sqrt(np.float64(N))
    wvc = wv.reshape(CI, P).T.astype(np.float32).copy()
    # W2[p, dc, c] = wk[dc*128+p] * wq[c] / sqrt(N)
    w2 = (wk.reshape(CI, P).T[:, :, None] * wq[None, None, :] / rn)
    w2 = w2.astype(np.float32).copy()
    return wvc, w2


def make_in_maps(x, wq, wk, wv):
    wvc, w2 = make_weight_inputs(wq, wk, wv)
    xr = np.ascontiguousarray(x.reshape(B_TOTAL, C, N))
    return [
        {
            "x": xr[core * B_PER_CORE:(core + 1) * B_PER_CORE],
            "wvc": wvc,
            "w2": w2,
        }
        for core in range(N_CORES)
    ]


def kernel(x: np.ndarray, wq: np.ndarray, wk: np.ndarray, wv: np.ndarray) -> np.ndarray:
    assert x.shape == (B_TOTAL, C, 64, 64) and x.dtype == np.float32
    nc = _get_nc()
    in_maps = make_in_maps(x, wq, wk, wv)
    res = run_bass_kernel_spmd(nc, in_maps, core_ids=list(range(N_CORES)))
    outs = [np.asarray(r["out"]).astype(np.float32) for r in res.results]
    return np.concatenate(outs, axis=0).reshape(B_TOTAL, C, 64, 64)


# revision 43
# speedup vs baseline: 1.0288x; 1.0288x over previous
"""Channel-attention (single-head shared attention over channels) Trainium2 kernel.

Reference computation (per batch b, C=512 channels, N=64*64=4096 spatial):
    xf = x[b].reshape(C, N)
    q = wq[:,None]*xf ; k = wk[:,None]*xf ; v = wv[:,None]*xf
    attn = softmax(q @ k.T / sqrt(N), axis=-1)        # (C, C)
    out[b] = (attn @ v).reshape(C, H, W)

Strategy (data-parallel over B across 8 cores, 2 batches/core):
  * Gram matrix in fp8 DoubleRow (0.5 cyc/row): x is cast RAW to fp8e4
    (values ~N(0,1) sit in e4m3's sweet spot; folding the tiny wq scale in,
    as a bf16 kernel would, lands in fp8 subnormals).  The transposed
    operands come from fp8 PE transposes (stride-2 PSUM writes per the ISA
    rule) compacted to stride-1 SBUF tiles laid out [n-pair, 2, C] so both
    DoubleRow APs are walrus-legal ([K, 2 (stride%16==0), M (stride 1)]).
  * Logits take the two-sided scale wk_d*wq_c/sqrt(N) from a host-computed
    rank-1 W2 matrix (DVE multiply) before the ACT exp -> E in bf16.
  * mm2 out = E @ v runs in float32r (full 1 cyc/row at 512-wide moving):
    the rhs is the raw staged f32 x (declared f32r, zero extra casts!) and
    the lhsT is E with wv folded per-partition (DVE, f32r out).  Z = sum E
    via ones-matmuls on the PE; the softmax 1/Z is applied as the per-
    partition scale of the PSUM->bf16 output downcast (ACT/DVE).
  * Output is written bf16 (halves the output DMA) and upcast on host.
  * Scheduling: engines execute strictly in-order, so PSUM-dependent ops
    (compaction) never share a queue with input-critical casts; batch 1's
    transposes+Gram are emitted inside batch 0's mm2 window; Z is emitted
    two mm2 groups in (the op-ring absorbs the downcast lag).
  Measured: 124413 ns TimelineSim (baseline 129427), rel err 2.3e-3.
"""

import numpy as np
import ml_dtypes

import concourse.bass as bass
import concourse.tile as tile
from concourse import mybir
from concourse.bass_utils import run_bass_kernel_spmd
from concourse.masks import make_identity

P = 128
C = 512
N = 4096
B_TOTAL = 16
N_CORES = 8
B_PER_CORE = B_TOTAL // N_CORES
CI = C // P          # 4 channel chunks
NCH = 8              # input staged in 512-n chunks
NPAIR = 16           # 256-n DoubleRow pairs per batch
F32 = mybir.dt.float32
F32R = mybir.dt.float32r
BF16 = mybir.dt.bfloat16
FP8 = mybir.dt.float8e4

MM2_FP8 = False      # bf16 mm2 (fp8 DoubleRow variant kept for reference)


def _split_multiwaits(nc):
    """Workaround: this walrus build rejects instructions carrying >1 sync
    wait ("Too many sync wait commands").  Hoist all but the last wait onto
    standalone EventSemaphore instructions placed just before the owner (same
    engine, so sequencer order preserves semantics)."""
    for f in nc.m.functions:
        for blk in f.blocks:
            new_insts = []
            for ins in blk.instructions:
                si = ins.sync_info
                if si is not None and si.on_wait is not None and len(si.on_wait) > 1:
                    waits = list(si.on_wait)
                    for k, w in enumerate(waits[:-1]):
                        new_insts.append(
                            mybir.InstEventSemaphore(
                                name=f"{ins.name}_splitw{k}",
                                engine=ins.engine,
                                sync_info=mybir.SyncInfo(on_wait=[w], on_update=[]),
                            )
                        )
                    si.on_wait = [waits[-1]]
                new_insts.append(ins)
            blk.instructions[:] = new_insts


def build_kernel():
    nc = bass.Bass()
    x_in = nc.dram_tensor("x", [B_PER_CORE, C, N], F32R, kind="ExternalInput")
    wvc_in = nc.dram_tensor("wvc", [P, CI], F32, kind="ExternalInput")
    w2_in = nc.dram_tensor("w2", [P, CI, C], F32, kind="ExternalInput")
    out = nc.dram_tensor("out", [B_PER_CORE, C, N], BF16, kind="ExternalOutput")

    with tile.TileContext(nc) as tc:
        with (
            tc.tile_pool(name="singles", bufs=1) as singles,
            
            tc.tile_pool(name="x8p", bufs=6) as x8p,
            tc.tile_pool(name="xtp", bufs=4) as xtp,
            tc.tile_pool(name="vp", bufs=2) as vp,
            tc.tile_pool(name="ep", bufs=2) as ep,
            tc.tile_pool(name="sp2", bufs=2) as sp2,
            tc.tile_pool(name="misc", bufs=4) as miscp,
            tc.tile_pool(name="rzp", bufs=8) as rzp,
            tc.tile_pool(name="osbp", bufs=4) as osbp,
            tc.tile_pool(name="gpp", bufs=4, space="PSUM") as gpp,
            tc.tile_pool(name="tpp", bufs=4, space="PSUM") as tpp,
        ):
            wvc = singles.tile([P, CI], F32, name="wvc")
            w2 = singles.tile([P, CI, C], F32, name="w2")
            nc.sync.dma_start(wvc, wvc_in[:, :])
            ident8 = singles.tile([P, P], FP8, name="ident8")
            make_identity(nc, ident8)
            identf = singles.tile([P, P], F32, name="identf")
            make_identity(nc, identf)
            onesb = singles.tile([P, 1], BF16, name="onesb")
            nc.gpsimd.memset(onesb, 1.0)

            # per-batch state (rings of 2)
            stgs, ets, etvs = {}, {}, {}
            x8s = {}          # (b, k) -> chunk tile
            xts = {}          # (b, k) -> transposed chunk tile
            gpss = {}         # b -> [gp tile per dc]
            rzs = {}          # (b, cc) -> 1/Z tile
            w2_loaded = [False]

            def nonpe_chunk(b, k, split_first=False):
                """Stage DMA + fp8/bf16 casts for chunk k of batch b."""
                nsl = slice(k * 512, (k + 1) * 512)
                xr = x_in[b].rearrange("(i p) n -> p i n", p=P)
                stg = stgs[b][:, :, nsl]
                x8 = x8p.tile([P, CI, 512], FP8, tag="x8", name=f"x8_{b}_{k}")
                x8s[(b, k)] = x8
                if split_first:
                    # first 256 n-cols land + cast early so the PE starts
                    # transposing ~2us sooner
                    nc.sync.dma_start(stg[:, :, :256], xr[:, :, 0:256])
                    nc.sync.dma_start(stg[:, :, 256:], xr[:, :, 256:512])
                    nc.gpsimd.tensor_copy(out=x8[:, :, :256], in_=stg[:, :, :256])
                    nc.gpsimd.tensor_copy(out=x8[:, :, 256:], in_=stg[:, :, 256:])
                elif k % 2 == 0:
                    nc.sync.dma_start(stg, xr[:, :, nsl])
                    nc.gpsimd.tensor_copy(out=x8[:, :, :], in_=stg[:, :, :])
                else:
                    nc.sync.dma_start(stg, xr[:, :, nsl])
                    nc.scalar.activation(
                        x8[:, :, :], stg[:, :, :],
                        func=mybir.ActivationFunctionType.Copy,
                    )
                if not w2_loaded[0]:
                    # W2 is first needed at phase3; load it after chunk 0
                    nc.sync.dma_start(w2, w2_in[:, :, :])
                    w2_loaded[0] = True

            def pe_tp_chunk(b, k):
                """fp8 transposes (stride-2 PSUM) + compaction for chunk k."""
                x8 = x8s.pop((b, k))
                xtu = xtp.tile([P, 2, 2, CI, P], FP8, tag="xtu", name=f"xtu_{b}_{k}")
                xts[(b, k)] = xtu
                for g2 in range(2):  # one 256-n pair per group
                    tpg = tpp.tile(
                        [P, 2048], FP8, tag="tpg", name=f"tpg_{b}_{k}_{g2}"
                    )
                    gsv = tpg.rearrange("p (q c two) -> p q c two", two=2, c=P)
                    for blk2 in range(2):
                        nb = g2 * 2 + blk2
                        for ci in range(CI):
                            nc.tensor.transpose(
                                gsv[:, blk2 * CI + ci, :, 0],
                                x8[:, ci, nb * P:(nb + 1) * P],
                                ident8,
                            )
                    # GPSIMD cannot read PSUM.  Batch 0's input phase is
                    # DMA-paced: DVE-only keeps ACT's x8 casts unblocked.
                    # Later batches run tp as a burst inside the previous
                    # mm2 window: split cmp across DVE+ACT (x8s are done).
                    cmp_out = xtu[:, g2].rearrange("p i ci c -> p (i ci c)")
                    cmp_in = gsv[:, :, :, 0].rearrange("p q c -> p (q c)")
                    if b == 0 or g2 == 0:
                        nc.vector.tensor_copy(out=cmp_out, in_=cmp_in)
                    else:
                        nc.scalar.activation(
                            cmp_out, cmp_in,
                            func=mybir.ActivationFunctionType.Copy,
                        )

            def pe_mm1_chunk(b, k):
                """DoubleRow Gram accumulation for the 2 pairs of chunk k.
                The last chunk runs dc-major so gp0 stops ~1us before gp3 and
                the exp chain overlaps the mm1 tail."""
                gps = gpss[b]
                xtu = xts.pop((b, k))
                if k == NCH - 1:
                    for dc in range(CI):
                        for g2 in range(2):
                            t = k * 2 + g2
                            _mm1_one(gps, xtu, g2, t, dc)
                    return
                for g2 in range(2):
                    t = k * 2 + g2
                    for dc in range(CI):
                        _mm1_one(gps, xtu, g2, t, dc)

            def _mm1_one(gps, xtu, g2, t, dc):
                if True:
                    if True:
                        lhsT = xtu[:, g2, :, dc, :]
                        rhs_all = xtu[:, g2].rearrange("p i ci c -> p i (ci c)")
                        off = 0
                        first_piece = True
                        while off < C:
                            w = min(256, C - off)
                            nc.tensor.matmul(
                                gps[dc][:, off:off + w],
                                lhsT=lhsT,
                                rhs=rhs_all[:, :, off:off + w],
                                start=(t == 0 and first_piece),
                                stop=(t == NPAIR - 1 and off + w == C),
                                perf_mode=mybir.MatmulPerfMode.DoubleRow,
                                skip_group_check=True,
                            )
                            off += w
                            first_piece = False
                        return

            def phase3(b):
                """W2 scale + exp (+mirrors), E splits, Z and reciprocals."""
                gps = gpss[b]
                et = ets[b]
                etv = etvs[b]
                for dc in range(CI):
                    s = sp2.tile([P, C], F32, tag="s", name=f"s_{b}_{dc}")
                    nc.vector.tensor_tensor(
                        out=s,
                        in0=gps[dc],
                        in1=w2[:, dc, :],
                        op=mybir.AluOpType.mult,
                    )
                    nc.scalar.activation(
                        et[:, dc, :], s,
                        func=mybir.ActivationFunctionType.Exp,
                    )
                    # fold wv into E per dc (Z uses the unfolded et tiles);
                    # f32r so the mm2 runs at full rate
                    nc.vector.tensor_scalar(
                        out=etv[:, dc, :], in0=et[:, dc, :],
                        scalar1=wvc[:, dc:dc + 1], scalar2=None,
                        op0=mybir.AluOpType.mult,
                    )
            def emit_z(b):
                """Z ones-matmuls + reciprocals, emitted a couple of mm2
                groups in so the PE never parks on the exp chain (the op
                ring absorbs the outscale lag)."""
                et = ets[b]
                for cc in range(CI):
                    zpt = gpp.tile([P, C], F32, tag="gp", name=f"zp_{b}_{cc}")
                    zp = zpt[:, 0:1]
                    for dc in range(CI):
                        nc.tensor.matmul(
                            zp,
                            lhsT=et[:, dc, cc * P:(cc + 1) * P],
                            rhs=onesb,
                            start=(dc == 0),
                            stop=(dc == CI - 1),
                        )
                    rz = rzp.tile([P, 1], F32, tag="rz", name=f"rz_{b}_{cc}")
                    rzs[(b, cc)] = rz
                    nc.vector.reciprocal(rz, zp)

            def mm2_group(b, cc, fine_tail=False, z_mid=False):
                """mm2 DoubleRow passes + 1/Z downcast + output DMA for cc."""
                csl = slice(cc * P, (cc + 1) * P)
                etv = etvs[b]
                xs = stgs[b]
                def one_nt_matmuls(nt):
                    op = gpp.tile([P, C], F32, tag="gp",
                                  name=f"op_{b}_{cc}_{nt}")[:, :512]
                    ntl = slice(nt * 512, (nt + 1) * 512)
                    for dc in range(CI):
                        nc.tensor.matmul(
                            op,
                            lhsT=etv[:, dc, csl],
                            rhs=xs[:, dc, ntl],
                            start=(dc == 0),
                            stop=(dc == CI - 1),
                        )
                    return op

                def one_nt_post(nt, op, osb, q):
                    # deferred softmax 1/Z as the PSUM->bf16 downcast
                    rz = rzs[(b, cc)]
                    if nt % 4 != 3:
                        nc.scalar.activation(
                            osb[:, q, :], op,
                            func=mybir.ActivationFunctionType.Copy,
                            scale=rz,
                        )
                    else:
                        nc.vector.tensor_scalar(
                            out=osb[:, q, :], in0=op,
                            scalar1=rz, scalar2=None,
                            op0=mybir.AluOpType.mult,
                        )
                    if fine_tail:
                        nc.scalar.dma_start(
                            out[b, csl, nt * 512:(nt + 1) * 512], osb[:, q, :]
                        )

                pending = []
                for nt2 in range(4):
                    osb = osbp.tile([P, 2, 512], BF16, tag="osb",
                                    name=f"osb_{b}_{cc}_{nt2}")
                    for q in range(2):
                        nt = nt2 * 2 + q
                        op = one_nt_matmuls(nt)
                        if z_mid and nt2 == 0:
                            # hold the downcasts until Z exists; the PE has
                            # these two groups as runway over the exp tail
                            pending.append((nt, op, osb, q))
                            continue
                        if z_mid and nt2 == 1 and pending:
                            emit_z(b)
                            for (pnt, pop, posb, pq) in pending:
                                one_nt_post(pnt, pop, posb, pq)
                            pending = []
                        one_nt_post(nt, op, osb, q)
                    if not fine_tail and not (z_mid and nt2 == 0):
                        nc.scalar.dma_start(
                            out[b, csl, nt2 * 1024:(nt2 + 1) * 1024], osb
                        )
                    elif not fine_tail and z_mid and nt2 == 0:
                        first_osb = osb
                if not fine_tail and z_mid:
                    nc.scalar.dma_start(out[b, csl, 0:1024], first_osb)

            # ---------------- emission schedule ----------------
            for b in range(B_PER_CORE):
                stgs[b] = vp.tile([P, CI, N], F32R, tag="stg", name=f"stg_{b}")
                ets[b] = ep.tile([P, CI, C], BF16, tag="et", name=f"et_{b}")
                etvs[b] = ep.tile([P, CI, C], F32R, tag="etv", name=f"etv_{b}")

                def alloc_gps():
                    gpss[b] = [
                        gpp.tile([P, C], F32, tag="gp", name=f"gp{dc}_{b}")
                        for dc in range(CI)
                    ]

                if b == 0:
                    alloc_gps()
                    for k in range(NCH):
                        nonpe_chunk(b, k, split_first=(k == 0))
                        pe_tp_chunk(b, k)
                        if k > 0:
                            pe_mm1_chunk(b, k - 1)
                    pe_mm1_chunk(b, NCH - 1)
                else:
                    # inputs of batch b interleave with mm2 of batch b-1;
                    # PE work (tp+mm1) is emitted after, so the tensor engine
                    # stream stays in-order and stall-free.
                    alloc_gps()
                    for cc in range(CI):
                        nonpe_chunk(b, 2 * cc)
                        nonpe_chunk(b, 2 * cc + 1)
                        mm2_group(b - 1, cc, z_mid=(cc == 0))
                        # fold this batch's transposes+Gram into the previous
                        # batch's mm2 window (PE stays busy either way; the
                        # dedicated tp phase vanishes from the critical path)
                        pe_tp_chunk(b, 2 * cc)
                        if cc > 0:
                            pe_mm1_chunk(b, 2 * cc - 1)
                        pe_tp_chunk(b, 2 * cc + 1)
                        pe_mm1_chunk(b, 2 * cc)
                    pe_mm1_chunk(b, NCH - 1)
                phase3(b)
            for cc in range(CI):
                mm2_group(B_PER_CORE - 1, cc, fine_tail=(cc >= 2), z_mid=(cc == 0))

    _split_multiwaits(nc)
    return nc


_NC_CACHE = None


def _get_nc():
    global _NC_CACHE
    if _NC_CACHE is None:
        _NC_CACHE = build_kernel()
    return _NC_CACHE


def make_weight_inputs(wq, wk, wv):
    wq = np.asarray(wq, np.float64)
    wk = np.asarray(wk, np.float64)
    wv = np.asarray(wv, np.float64)
    rn = np.sqrt(np.float64(N))
    wvc = wv.reshape(CI, P).T.astype(np.float32).copy()
    # W2[p, dc, c] = wk[dc*128+p] * wq[c] / sqrt(N)
    w2 = (wk.reshape(CI, P).T[:, :, None] * wq[None, None, :] / rn)
    w2 = w2.astype(np.float32).copy()
    return wvc, w2


def make_in_maps(x, wq, wk, wv):
    wvc, w2 = make_weight_inputs(wq, wk, wv)
    xr = np.ascontiguousarray(x.reshape(B_TOTAL, C, N))
    return [
        {
            "x": xr[core * B_PER_CORE:(core + 1) * B_PER_CORE],
            "wvc": wvc,
            "w2": w2,
        }
        for core in range(N_CORES)
    ]


def kernel(x: np.ndarray, wq: np.ndarray, wk: np.ndarray, wv: np.ndarray) -> np.ndarray:
    assert x.shape == (B_TOTAL, C, 64, 64) and x.dtype == np.float32
    nc = _get_nc()
    in_maps = make_in_maps(x, wq, wk, wv)
    res = run_bass_kernel_spmd(nc, in_maps, core_ids=list(range(N_CORES)))
    outs = [np.asarray(r["out"]).astype(np.float32) for r in res.results]
    return np.concatenate(outs, axis=0).reshape(B_TOTAL, C, 64, 64)
